# revision 4
# baseline (speedup 1.0000x reference)
"""Trainium2 Bass kernel for nn_AttnLayer (additive attention over history).

Math (per batch b, S = T*N = 8192 positions, A = H = 128):
    c[b]      = cur_h[b] @ Wx_w.T + Wx_b + Wh_b                  (host, tiny)
    pj[a,s]   = alpha * (sum_h Wh[a,h] hist[s,h] + c[b,a])       (PE, [a,s] layout)
    tnh[a,s]  = tanh(pj/alpha)   ACT share: native tanh (scale arg is free)
                                 DVE share: custom fused 7-stage op = clamped
                                 odd deg-5 poly z(TK0+q(TK1+q)), q=z^2,
                                 z = clamp(pj, +-BZ)   (|err| <= 1.7e-2)
    score[s]  = sum_a v[a] tnh[a,s]    (PE matvec: tnh chunk stationary, v moving)
    esc       = exp(score)  (bf16)     (ACT; accum_out gives per-partition
                                        partial sums of esc -> z on host)
    attn_h[h] = (sum_s esc[s] hist[s,h]) / z
                (PE pass-2: histN tile stationary, esc column moving, ONE
                 psum accumulation chain per batch, emitted AFTER the whole
                 pj/mv stream so the in-order PE queue never head-blocks)
    out[b]    = cur_h[b] + attn_h                                (host, tiny)

Layouts (host pre-packed, all history fp8 e4m3):
    histT8[b][p][j*8192+s] = hist[b,s,64j+p], row 64 = ones   pass-1 moving
        (DoubleRow: contraction 64 partitions x 2; the ones row pairs with a
         fp8 coarse+residual alpha*c bias pair in the stationary row 64, so
         the bias costs zero PE cycles)
    histN8[b][p][i*128+h]  = hist[b,128i+p,h]                 pass-2 stationary

Schedule (v2, DMA-stream-shaped): the wire is the critical resource
(~24.0 us of fp8 transfers at 360 GB/s). Wire order: w8, v16, small histT
lead piece (so PE starts ~2.4us in), then ALL remaining histT (so the
tanh engines are never starved and finish with the stream), then ALL
histN, then the 4 tiny output DMAs. Pass-2 + fin are emitted behind the
full pj/mv stream; only the last batch's second p2 run + fin + out DMA
trail the final input byte (~4us structural tail: 0.9us DMA-sem prop,
p2 run, fin copy, HWDGE+DGE issue, 0.9us out-sem prop, drain).

Sharding: data-parallel over batch B=32 across 8 cores (4 batches/core).
"""

import os
import sys
from contextlib import ExitStack

import numpy as np
import ml_dtypes

for _p in (
    "/root/.axon_site",
    "/root/.axon_site/_ro/trn_rl_repo",
    "/root/.axon_site/_ro/pypackages",
    "/opt/trn_rl_repo",
):
    if os.path.isdir(_p) and _p not in sys.path:
        sys.path.append(_p)

import concourse.bass as bass  # noqa: E402
import concourse.tile as tile  # noqa: E402
from concourse import bacc, mybir  # noqa: E402
import concourse.bass_utils as bass_utils  # noqa: E402
import concourse.dve_ops as dve_ops  # noqa: E402
from concourse.dve_spec import (  # noqa: E402
    Spec, Src0, Src1, C0, C1, C2, maxx, minn, lower, _has_src1,
)
from concourse.dve_uop import DveOpSpec  # noqa: E402
from concourse.dve_table_gen import dve_ver_for  # noqa: E402

BF16 = mybir.dt.bfloat16
FP8 = mybir.dt.float8e4
F32 = mybir.dt.float32
NPBF16 = ml_dtypes.bfloat16
NPFP8 = ml_dtypes.float8_e4m3

B, T, N, HID, ATTN = 32, 64, 128, 128, 128
NCORES = 8
BL = B // NCORES          # batches per core
S = T * N                 # history positions per batch (8192)
P = 128
HP = 64                   # half partitions (DoubleRow contraction = 64 x 2)
KC = 512                  # chunk columns (1 psum bank; tanh instruction size)
NKC = S // KC             # chunks per batch (16)
LEAD = int(os.environ.get("K_LEAD", "1024"))  # batch-0 lead piece (s cols)
PJB = int(os.environ.get("K_PJB", "5"))   # pj psum buffers (banks)
# engine plan per chunk position: 'A' = ACT tanh, 'D' = DVE poly tanh
PLAN = os.environ.get("K_PLAN", "DA" * 32)
LAG = int(os.environ.get("K_LAG", "4"))
NT = S // P               # pass-2 s-tiles per batch (64)
HN = NT * P               # histN8 bytes/partition ([128, 8192])
NHALF = 2                 # histN DMA halves per batch

# clamped odd deg-5 tanh fit (z = ALPHA*x clamped to +-BZ):
# tanh(x) ~= z*(TK0 + q*(TK1 + q)), q = z*z;  max abs err 1.61e-2
ALPHA = 0.447118
TK0 = 2.107214
TK1 = -2.107472
BZ = 0.983659

_cache = {}
TASKLOG = []   # (n_instructions_at_mark, label) for trace attribution


def _mark(tc, label):
    try:
        TASKLOG.append((int(tc.nc.next_id()), label))
    except Exception:
        pass


def _register_tanh5():
    """Register the fused clamp+poly tanh DVE op (7 ALU stages, 1 uop)."""
    name = "TANH5_CLAMP_ANT"
    for op in dve_ops.OPS:
        if op.name == name:
            return op
    z = minn(maxx(Src0, C0), C1)
    q = z * z
    body = ((q + C2) * q + Src1) * z

    def ref(in0, in1, c0, c1, c2):
        zz = np.minimum(np.maximum(in0.astype(np.float32), c0), c1)
        qq = zz * zz
        return ((qq + c2) * qq + in1) * zz

    spec = Spec(body=body, reference=ref)
    ver = dve_ver_for("TRN2")
    free = [r for r in range(1, 32) if r not in dve_ops._SUB_OPCODE_FOR_NAME.values()]
    row = free[0]
    s = DveOpSpec(name=name, opcode=row, uops=lower(spec, ver=ver),
                  rd1_en=_has_src1(spec))
    op = dve_ops.DveOp(name, spec, subdim=False, uops_sha={ver: s.sha(ver)})
    dve_ops.OPS.append(op)
    dve_ops._SUB_OPCODE_FOR_NAME[name] = row
    dve_ops.CUSTOM_DVE_SPECS[name] = spec
    return op


TANH5 = _register_tanh5()


def _build_kernel(tc, histT8, histN8, wpack8, v16, acc_out):
    nc = tc.nc
    AF = mybir.ActivationFunctionType
    DR = mybir.MatmulPerfMode.DoubleRow
    with ExitStack() as ctx:
        wpool = ctx.enter_context(tc.tile_pool(name="w", bufs=1))
        bigT = ctx.enter_context(tc.tile_pool(name="bigT", bufs=2 * BL + 1))
        bigN = ctx.enter_context(tc.tile_pool(name="bigN", bufs=NHALF * BL))
        pjp = ctx.enter_context(tc.tile_pool(name="pj", bufs=PJB, space="PSUM"))
        sap = ctx.enter_context(tc.tile_pool(name="sa", bufs=2, space="PSUM"))
        azp = ctx.enter_context(tc.tile_pool(name="az", bufs=1, space="PSUM"))
        tnhp = ctx.enter_context(tc.tile_pool(name="tnh", bufs=int(os.environ.get("K_TNB", "10"))))
        escp = ctx.enter_context(tc.tile_pool(name="esc", bufs=BL))
        obp = ctx.enter_context(tc.tile_pool(name="ob", bufs=BL))

        # --- weights + lead piece first on the sync ring: the wire starts
        # with the three tiny transfers the first pj matmul needs ---
        w8 = wpool.tile([HP + 1, BL * 2 * P], FP8, tag="w8")
        _mark(tc, "loadW")
        nc.sync.dma_start(w8[:], wpack8)
        wbs = [
            w8[:, 2 * P * b : 2 * P * (b + 1)].rearrange("p (two m) -> p two m", two=2)
            for b in range(BL)
        ]
        vsb = wpool.tile([P, 1], BF16, tag="v16")
        nc.sync.dma_start(vsb[:], v16)
        k0t = wpool.tile([P, KC], F32, tag="k0")
        nc.gpsimd.memset(k0t[:], TK0)

        # --- history loads: histT pieces (lead + rest) strictly before all
        # histN halves, so the tanh stream is never feed-starved and only
        # pass-2 (cheap on PE) trails the wire ---
        Tbs = {b: [] for b in range(BL)}
        Nbs = {b: [] for b in range(BL)}
        PIECES = ([[LEAD, S // 2 - LEAD, S // 2]]
                  + [[S // 2, S // 2]] * (BL - 1))

        def load_T(b, q):
            _mark(tc, f"loadT({b},{q})")
            ps = PIECES[b][q]
            off = sum(PIECES[b][:q])
            t = bigT.tile([HP + 1, 2 * ps], FP8, tag="histT",
                          name=f"histT{b}_{q}")
            src = histT8[b].rearrange("p (two s) -> p two s", two=2)
            nc.sync.dma_start(
                t[:].rearrange("p (two s) -> p two s", two=2),
                src[:, :, off : off + ps],
            )
            Tbs[b].append((t, ps))

        def load_N(b, q):
            _mark(tc, f"loadN({b},{q})")
            t = bigN.tile([P, HN // NHALF], FP8, tag="histN")
            nc.sync.dma_start(
                t[:], histN8[b][:, (HN // NHALF) * q : (HN // NHALF) * (q + 1)]
            )
            Nbs[b].append(t)

        for b in range(BL):
            for q in range(len(PIECES[b])):
                load_T(b, q)
        for b in range(BL):
            for q in range(NHALF):
                load_N(b, q)

        def histT_slice(b, s0, ncols):
            """[64, 2, ncols] moving slice for s-range [s0, s0+ncols)."""
            for t, piece_s in Tbs[b]:
                if s0 < piece_s:
                    ap = t[:].rearrange("p (two s) -> p two s", two=2)
                    return ap[:, :, s0 : s0 + ncols]
                s0 -= piece_s
            raise AssertionError("bad slice")

        scoreaccs = {}
        tnhs = {}
        escs = {}
        obs = {}

        def prod(b, kc, eng):
            """pass-1 chunk: one DoubleRow matmul (bias in row 64) + tanh."""
            _mark(tc, f"prod{eng}({b},{kc})")
            pj = pjp.tile([P, KC], F32, tag="pj")
            nc.tensor.matmul(
                pj[:],
                wbs[b],
                histT_slice(b, KC * kc, KC),
                start=True, stop=True,
                perf_mode=DR,
            )
            tnh = tnhp.tile([P, KC], BF16, tag="tnh")
            if eng == "D":
                nc.vector._custom_dve(
                    TANH5, out=tnh[:], in0=pj[:], in1=k0t[:],
                    s0=-BZ, s1=BZ, imm2=TK1,
                )
            else:
                nc.scalar.activation(
                    tnh[:], pj[:], AF.Tanh, scale=1.0 / ALPHA,
                )
            tnhs[(b, kc)] = tnh

        def matvecs(b, kc):
            """score columns for chunk kc: 4 matvecs, out [128,1] each."""
            if kc == 0:
                scoreaccs[b] = sap.tile([P, NT], F32, tag="sa", name=f"sa{b}")
            sa = scoreaccs[b]
            tnh = tnhs.pop((b, kc))
            _mark(tc, f"mv({b},{kc})")
            for m in range(KC // P):
                i = (KC // P) * kc + m           # s-tile index
                nc.tensor.matmul(
                    sa[:, i : i + 1],
                    tnh[:, P * m : P * (m + 1)],
                    vsb[:],
                    start=True, stop=True,
                )

        def exp_task(b):
            """exp of the batch's scores -> bf16 esc; accum_out = esc
            per-partition partial sums -> ob col 1 (z finished on host)."""
            sa = scoreaccs.pop(b)
            _mark(tc, f"exp({b})")
            esc = escp.tile([P, NT], BF16, tag="esc", name=f"esc{b}")
            escs[b] = esc
            ob = obp.tile([P, 2], F32, tag="ob", name=f"ob{b}")
            obs[b] = ob
            nc.scalar.activation(esc[:], sa[:, 0:NT], AF.Exp,
                                 accum_out=ob[:, 1:2])

        az = None

        def p2_run(b, q):
            """pass-2 half: 32 stationary histN tiles x esc column moving,
            one psum accumulation chain per batch (az col b)."""
            nonlocal az
            _mark(tc, f"p2({b},{q})")
            if az is None:
                az = azp.tile([P, BL], F32, tag="az")
            esc = escs[b]
            nb = Nbs[b][q]
            hs = NT // NHALF
            for i in range(hs * q, hs * (q + 1)):
                nc.tensor.matmul(
                    az[:, b : b + 1],
                    nb[:, P * (i % hs) : P * (i % hs + 1)],
                    esc[:, i : i + 1],
                    start=(i == 0), stop=(i == NT - 1),
                )

        def fin_task(b):
            """attn numerator psum -> ob col 0 (ACT copy) + output DMA."""
            _mark(tc, f"fin({b})")
            escs.pop(b)
            ob = obs.pop(b)
            nc.scalar.copy(ob[:, 0:1], az[:, b : b + 1])
            nc.sync.dma_start(acc_out[b], ob[:])

        # --- main stream: strict chunk order, mv lagged LAG producers
        # behind, exp right after its last mv ---
        chunk_seq = [(b, kc) for b in range(BL) for kc in range(NKC)]
        emitted = 0

        def drain_consumers(upto):
            nonlocal emitted
            while emitted < upto:
                cb, ckc = chunk_seq[emitted]
                matvecs(cb, ckc)
                emitted += 1
                if ckc == NKC - 1:
                    exp_task(cb)

        for i, (b, kc) in enumerate(chunk_seq):
            prod(b, kc, PLAN[i % len(PLAN)])
            drain_consumers(i + 1 - LAG)
        drain_consumers(len(chunk_seq))

        # --- endgame: pass-2 + fin, all behind the pj/mv stream in the
        # in-order PE queue (histN data races only the wire, not compute) ---
        for b in range(BL):
            for q in range(NHALF):
                p2_run(b, q)
            fin_task(b)


def build():
    if "nc" in _cache:
        return _cache["nc"]
    nc = bacc.Bacc(
        "TRN2",
        target_bir_lowering=False,
        debug=False,
        enable_asserts=True,
        num_devices=NCORES,
    )
    histT8 = nc.dram_tensor("histT8", [BL, HP + 1, 2 * S], FP8, kind="ExternalInput").ap()
    histN8 = nc.dram_tensor("histN8", [BL, P, HN], FP8, kind="ExternalInput").ap()
    wpack8 = nc.dram_tensor("wpack8", [HP + 1, BL * 2 * P], FP8, kind="ExternalInput").ap()
    v16 = nc.dram_tensor("v16", [P, 1], BF16, kind="ExternalInput").ap()
    acc_out = nc.dram_tensor("acc_out", [BL, P, 2], F32, kind="ExternalOutput").ap()

    with tile.TileContext(nc) as tc:
        _build_kernel(tc, histT8, histN8, wpack8, v16, acc_out)
    nc.compile()
    _cache["nc"] = nc
    return nc


def make_in_maps(cur_h, history_h, Wx_w, Wx_b, Wh_w, Wh_b, v_w):
    """Host-side prep: shard over batch, pre-pack fp8 layouts, fold tiny ops."""
    cur_h = np.asarray(cur_h, np.float32)
    hist = np.asarray(history_h, np.float32).reshape(B, S, HID)
    c = (cur_h @ np.asarray(Wx_w, np.float32).T
         + np.asarray(Wx_b, np.float32)
         + np.asarray(Wh_b, np.float32))                      # [B, A]

    # pass-1 moving: histT8[b, p, j*S + s] = hist[b, s, 64j+p]; row 64 = ones
    hT = np.ascontiguousarray(hist.transpose(0, 2, 1))        # [B, H, S]
    histT8 = np.ones((B, HP + 1, 2 * S), NPFP8)
    histT8[:, :HP] = (hT.reshape(B, 2, HP, S).transpose(0, 2, 1, 3)
                      .reshape(B, HP, 2 * S).astype(NPFP8))
    # pass-2 stationary: histN8[b, p, i*128 + h] = hist[b, 128i+p, h]
    histN8 = np.ascontiguousarray(
        hist.reshape(B, NT, P, HID).transpose(0, 2, 1, 3)
    ).reshape(B, P, HN).astype(NPFP8)

    whT = np.asarray(Wh_w, np.float32).T * ALPHA              # [h, a] scaled
    whT8 = (whT.reshape(2, HP, ATTN).transpose(1, 0, 2)
            .reshape(HP, 2 * ATTN)).astype(NPFP8)             # [64, 2*128]

    v16 = np.ascontiguousarray(np.asarray(v_w, np.float32)[:, None]).astype(NPBF16)

    in_maps = []
    for qq in range(NCORES):
        bsl = slice(BL * qq, BL * (qq + 1))
        cq = c[bsl] * ALPHA                                   # [BL, A]
        # wpack8 is the exact SBUF image: [65, BL*2*128], per-batch slab
        # [65, 2, 128] = (whT8 rows 0-63, row 64 = coarse|resid bias pair)
        wpack8 = np.zeros((HP + 1, BL * 2 * ATTN), NPFP8)
        coarse = cq.astype(NPFP8)                             # bias row, j=0
        resid = (cq - coarse.astype(np.float32)).astype(NPFP8)  # j=1
        for b in range(BL):
            wpack8[:HP, 2 * ATTN * b : 2 * ATTN * (b + 1)] = whT8
            wpack8[HP, 2 * ATTN * b : 2 * ATTN * b + ATTN] = coarse[b]
            wpack8[HP, 2 * ATTN * b + ATTN : 2 * ATTN * (b + 1)] = resid[b]
        in_maps.append(
            {
                "histT8": np.ascontiguousarray(histT8[bsl]),
                "histN8": np.ascontiguousarray(histN8[bsl]),
                "wpack8": wpack8,
                "v16": v16,
            }
        )
    return in_maps, cur_h


def finish_host(results, cur):
    outs = []
    for q in range(NCORES):
        acc = results[q]["acc_out"]          # [BL, P, 2]: attn_num | z parts
        z = acc[:, :, 1].sum(axis=1, keepdims=True)
        outs.append(acc[:, :, 0] / z)
    attn = np.concatenate(outs, axis=0)
    return (cur + attn).astype(np.float32)


def kernel(cur_h, history_h, Wx_w, Wx_b, Wh_w, Wh_b, v_w):
    nc = build()
    in_maps, cur = make_in_maps(cur_h, history_h, Wx_w, Wx_b, Wh_w, Wh_b, v_w)
    res = bass_utils.run_bass_kernel_spmd(nc, in_maps, core_ids=list(range(NCORES)))
    return finish_host(res.results, cur)


if __name__ == "__main__":
    build()
    print("build ok")
